# revision 39
# baseline (speedup 1.0000x reference)
"""BondMessagePassing kernel for 8 Trainium2 NeuronCores.

Target-window edge sharding: core c owns the 512 edges whose target node
lies in [128c, 128(c+1)), so the segment-sum S and the gather S[tgt] are
fully local (no AllReduce). Per layer:
  - S via one-hot matmul over the local 512 edges -> 128 nodes
  - r = S[tgt] - deg*h fused on DVE
  - full-sequence MHA over 4096 edges in fp8: Q/K/V quantized to fp8e4,
    scores exp'd via the Schraudolph bit trick (round(s*c1+c2) as int8
    bitcast to fp8e4) split across the ACT and DVE engines, probabilities
    consumed by fp8 PV matmuls -- DoubleRow (2 col/cycle) for shard pairs
    (j, j+4), plain fp8 for the local shard (overlapped with the K/V
    AllGathers) and shard 4.
K and V_aug are AllGathered in fp8 as two collectives so QK on remote
shards can start before V lands. The final output is un-permuted on host.
"""

import numpy as np
import ml_dtypes

import concourse.bass as bass
import concourse.tile as tile
import concourse.mybir as mybir
from concourse import bacc
from concourse.bass_utils import run_bass_kernel_spmd
from concourse.masks import make_identity

F32 = mybir.dt.float32
BF16 = mybir.dt.bfloat16
FP8 = mybir.dt.float8e4
I8 = mybir.dt.int8
AF = mybir.ActivationFunctionType
ALU = mybir.AluOpType
BFNP = ml_dtypes.bfloat16
PM_DR = mybir.MatmulPerfMode.DoubleRow

NC = 8          # cores
P = 128         # partitions
NN = 1024       # nodes
E = 4096        # edges
EL = E // NC    # edges per core (512)
H = 256         # hidden
BD = 64         # bond dim
NH = 8          # heads
D = H // NH     # head dim (32)
L = 3           # layers
HK = H // P     # 2  K-chunks per 256
EC = EL // P    # 4  edge chunks per core
M6 = 3 * H // P  # 6 qkv out tiles
NW = NN // NC   # 128 nodes per core (target window)

# Schraudolph exp -> fp8e4 bits: bits = round(s * SCC1 + SCC2)
# (1/sqrt(d) folded into SCC1; tuned offset SCC2)
LOG2E = 1.4426950408889634
SCC1 = 8.0 * LOG2E / float(np.sqrt(np.float32(D)))
SCC2 = 55.7

KB_K = HK * EL            # 1024 fp8 bytes/partition of K^T payload
KB_V = EC * NH * 33       # 1056 fp8 bytes/partition of V_aug payload


def _build():
    nc = bacc.Bacc(None, target_bir_lowering=False, num_devices=NC)

    di = {}
    def din(name, shape, dtype):
        di[name] = nc.dram_tensor(name, shape, dtype, kind="ExternalInput")
        return di[name]

    din("bondT", [BD, EL], BF16)
    din("Amat", [P, EC, NW], BF16)
    din("Bmat", [P, EC, P], BF16)
    din("negdeg", [P, EC], F32)
    din("wemb", [BD, H], BF16)
    din("bemb", [P, HK], F32)
    din("wh", [P, HK, H], BF16)
    din("bh", [P, HK], F32)
    din("inw", [P, L, HK, 3 * H], BF16)
    din("inb", [P, L, M6], F32)
    din("outw", [P, L, HK, H], BF16)
    din("upw", [P, L, HK, H], BF16)
    din("upb2", [L, H], F32)
    din("ln2g", [L, H], F32)
    din("ln2b", [L, H], F32)
    din("inbv", [L, H], F32)
    din("qmask", [P, 2], F32)
    din("inbqm", [P, L, HK, 2], F32)
    din("bselp", [P, 4, P], BF16)
    hout = nc.dram_tensor("hout", [EL, H], F32, kind="ExternalOutput")

    rg = [list(range(NC))]

    with tile.TileContext(nc) as tc:
        with (
            tc.tile_pool(name="const", bufs=1) as const,
            tc.tile_pool(name="sb", bufs=2) as sb,
            tc.tile_pool(name="kv", bufs=1) as kv,
            tc.tile_pool(name="ptp", bufs=6) as ptp,
            tc.tile_pool(name="pmm2", bufs=2, space="PSUM") as pmm2,
            tc.tile_pool(name="pacc", bufs=4, space="PSUM") as pacc,
            tc.tile_pool(name="dram", bufs=1, space="DRAM") as dram,
        ):
            # ---- load constants ----
            bondT_sb = const.tile([BD, EL], BF16)
            nc.sync.dma_start(bondT_sb[:], di["bondT"][:])
            A_sb = const.tile([P, EC, NW], BF16)
            nc.sync.dma_start(A_sb[:], di["Amat"][:])
            B_sb = const.tile([P, EC, P], BF16)
            nc.sync.dma_start(B_sb[:], di["Bmat"][:])
            negdeg_sb = const.tile([P, EC], F32)
            nc.sync.dma_start(negdeg_sb[:], di["negdeg"][:])
            wemb_sb = const.tile([BD, H], BF16)
            nc.sync.dma_start(wemb_sb[:], di["wemb"][:])
            bemb_sb = const.tile([P, HK], F32)
            nc.sync.dma_start(bemb_sb[:], di["bemb"][:])
            wh_sb = const.tile([P, HK, H], BF16)
            nc.sync.dma_start(wh_sb[:], di["wh"][:])
            bh_sb = const.tile([P, HK], F32)
            nc.sync.dma_start(bh_sb[:], di["bh"][:])
            inw_sb = const.tile([P, L, HK, 3 * H], BF16)
            nc.sync.dma_start(inw_sb[:], di["inw"][:])
            inb_sb = const.tile([P, L, M6], F32)
            nc.sync.dma_start(inb_sb[:], di["inb"][:])
            outw_sb = const.tile([P, L, HK, H], BF16)
            nc.sync.dma_start(outw_sb[:], di["outw"][:])
            upw_sb = const.tile([P, L, HK, H], BF16)
            nc.sync.dma_start(upw_sb[:], di["upw"][:])

            def bcast_load(name):
                t = const.tile([P, L, H], F32, name=f"{name}_bc")
                src = di[name][:]
                bap = bass.AP(
                    tensor=src.tensor,
                    offset=src.offset,
                    ap=[[0, P]] + [list(x) for x in src.ap],
                )
                nc.sync.dma_start(t[:], bap)
                return t

            upb2_bc = bcast_load("upb2")
            ln2g_bc = bcast_load("ln2g")
            ln2b_bc = bcast_load("ln2b")
            inbv_bc = bcast_load("inbv")

            ident_bf = const.tile([P, P], BF16)
            make_identity(nc, ident_bf[:])
            eps_sb = const.tile([P, 1], F32)
            nc.vector.memset(eps_sb[:], 1e-5)
            sc2_sb = const.tile([P, 1], F32)
            nc.vector.memset(sc2_sb[:], SCC2)
            zero_sb = const.tile([P, 1], F32)
            nc.vector.memset(zero_sb[:], 0.0)
            # parity masks for the Q zero-slot trick: mask[v][p] = 1 iff
            # (p%32)//16 == v (host-filled)
            mask_sb = const.tile([P, 2], F32)
            nc.sync.dma_start(mask_sb[:], di["qmask"][:])
            inbqm_sb = const.tile([P, L, HK, 2], F32)
            nc.sync.dma_start(inbqm_sb[:], di["inbqm"][:])
            bselp_sb = const.tile([P, 4, P], BF16)
            nc.sync.dma_start(bselp_sb[:], di["bselp"][:])

            def transpose_128(dst_ap, src_ap):
                pst = pmm2.tile([P, P], BF16, tag="mm", name="pst")
                nc.tensor.transpose(pst[:], src_ap, ident_bf[:])
                nc.scalar.activation(dst_ap, pst[:], AF.Identity, bias=zero_sb[:])

            # ---- embedding: h = gelu(bond @ W_emb + b_emb) @ W_h + b_h ----
            g1 = sb.tile([P, HK, EL], BF16, name="g1")
            for m in range(HK):
                ps = pmm2.tile([P, 2, EL], F32, tag="mm", name="ps_e")
                nc.tensor.matmul(
                    ps[:, 0, :], wemb_sb[:, m * P:(m + 1) * P], bondT_sb[:],
                    start=True, stop=True,
                )
                nc.scalar.activation(
                    g1[:, m, :], ps[:, 0, :], AF.Gelu, bias=bemb_sb[:, m:m + 1]
                )
            hT = sb.tile([P, HK, EL], BF16, name="hT")
            for m in range(HK):
                ps = pmm2.tile([P, 2, EL], F32, tag="mm", name="ps_h")
                for k in range(HK):
                    nc.tensor.matmul(
                        ps[:, 0, :], wh_sb[:, k, m * P:(m + 1) * P], g1[:, k, :],
                        start=(k == 0), stop=(k == HK - 1),
                    )
                nc.vector.tensor_scalar_add(hT[:, m, :], ps[:, 0, :], bh_sb[:, m:m + 1])
            h_nat = sb.tile([P, EC, H], BF16, name="h_nat")
            for m in range(HK):
                for c in range(EC):
                    transpose_128(
                        h_nat[:, c, m * P:(m + 1) * P],
                        hT[:, m, c * P:(c + 1) * P],
                    )

            me = nc.sync.partition_id()

            # ---- layers ----
            for t in range(L):
                # A. local segment-sum over the 512 edges targeting this
                # core's 128-node window: S [128 nodes, H]
                psS = pmm2.tile([P, 2, EL], F32, tag="mm", name="psS")
                for c in range(EC):
                    nc.tensor.matmul(
                        psS[:, 0, :H], A_sb[:, c, :], h_nat[:, c, :],
                        start=(c == 0), stop=(c == EC - 1),
                    )
                s_bf = sb.tile([P, H], BF16, name="s_bf")
                nc.scalar.activation(s_bf[:], psS[:, 0, :H], AF.Identity, bias=zero_sb[:])

                # B. r = S[tgt] - deg[tgt]*h: local gather via matmul,
                # diagonal term fused on DVE
                r_nat = sb.tile([P, EC, H], F32, name="r_nat")
                mv4 = sb.tile([P, EC, 2], F32, name="mv4")
                for m in range(EC):
                    ps = pmm2.tile([P, 2, EL], F32, tag="mm", name="ps_r")
                    nc.tensor.matmul(
                        ps[:, 0, :H], B_sb[:, m, :], s_bf[:],
                        start=True, stop=True,
                    )
                    nc.vector.scalar_tensor_tensor(
                        r_nat[:, m, :], h_nat[:, m, :], negdeg_sb[:, m:m + 1],
                        ps[:, 0, :H], op0=ALU.mult, op1=ALU.add,
                    )
                    stats = sb.tile([P, 6], F32, tag="stats", name="stats")
                    nc.vector.bn_stats(stats[:], r_nat[:, m, :])
                    nc.vector.bn_aggr(mv4[:, m, :], stats[:])
                # C. LN1 -> xn (bf16); ln1 gamma/beta folded into in_w/in_b
                rstd4 = sb.tile([P, EC], F32, name="rstd4")
                nc.scalar.activation(rstd4[:], mv4[:, :, 1], AF.Sqrt, bias=eps_sb[:])
                nc.vector.reciprocal(rstd4[:], rstd4[:])
                negm4 = sb.tile([P, EC], F32, name="negm4")
                nc.vector.scalar_tensor_tensor(
                    negm4[:], mv4[:, :, 0], -1.0, rstd4[:],
                    op0=ALU.mult, op1=ALU.mult,
                )
                xn_bf = sb.tile([P, EC, H], BF16, name="xn_bf")
                for m in range(EC):
                    nc.scalar.activation(
                        xn_bf[:, m, :], r_nat[:, m, :], AF.Identity,
                        scale=rstd4[:, m:m + 1], bias=negm4[:, m:m + 1],
                    )

                # D. xn^T
                xnT = sb.tile([P, HK, EL], BF16, name="xnT")
                for c in range(EC):
                    for hf in range(HK):
                        transpose_128(
                            xnT[:, hf, c * P:(c + 1) * P],
                            xn_bf[:, c, hf * P:(hf + 1) * P],
                        )

                # E. in-proj K first (feeds AG1), then V (AG2), then Q.
                # Q/K in DoubleRow group layout (host-permuted columns):
                # partition p = slot*32 + parity*16 + d holds head 2*slot+
                # parity, dim g*16+d in group slice g. K bias dropped
                # (constant over keys under softmax); Q bias kept.
                KTl = sb.tile([P, HK, EL], FP8, name="KTl")
                for g in range(HK):
                    ps = pmm2.tile([P, 2, EL], F32, tag="mm", name="ps_k")
                    for k in range(HK):
                        nc.tensor.matmul(
                            ps[:, 0, :], inw_sb[:, t, k, (2 + g) * P:(3 + g) * P],
                            xnT[:, k, :],
                            start=(k == 0), stop=(k == HK - 1),
                        )
                    nc.scalar.activation(
                        KTl[:, g, :], ps[:, 0, :], AF.Identity, bias=zero_sb[:]
                    )
                ag_inK = dram.tile([P, KB_K], FP8, name=f"ag_inK{t}")
                ag_outK = dram.tile(
                    [P * NC, KB_K], FP8, addr_space="Shared", name=f"ag_outK{t}"
                )
                nc.sync.dma_start(
                    ag_inK[:].rearrange("p (a b) -> p a b", a=HK), KTl[:]
                )
                nc.gpsimd.collective_compute(
                    "AllGather", ALU.bypass, replica_groups=rg,
                    ins=[ag_inK[:]], outs=[ag_outK[:]],
                )

                # V_aug in fp8: vnat[c, h, 0:32]=v, [...,32]=1.0
                vnat = kv.tile([P, EC, NH, 33], FP8, name="vnat", tag="v0")
                nc.vector.memset(vnat[:, :, :, 32:33], 1.0)
                for c in range(EC):
                    ps = pmm2.tile([P, 2, EL], F32, tag="mm", name="ps_v")
                    for k in range(HK):
                        nc.tensor.matmul(
                            ps[:, 0, :H], xnT[:, k, c * P:(c + 1) * P],
                            inw_sb[:, t, k, 2 * H:3 * H],
                            start=(k == 0), stop=(k == HK - 1),
                        )
                    nc.vector.tensor_add(
                        vnat[:, c, :, 0:32],
                        ps[:, 0, :H].rearrange("p (a b) -> p a b", a=NH),
                        inbv_bc[:, t, :].rearrange("p (a b) -> p a b", a=NH),
                    )
                ag_inV = dram.tile([P, KB_V], FP8, name=f"ag_inV{t}")
                ag_outV = dram.tile(
                    [P * NC, KB_V], FP8, addr_space="Shared", name=f"ag_outV{t}"
                )
                nc.sync.dma_start(
                    ag_inV[:].rearrange("p (a b c) -> p a b c", a=EC, b=NH),
                    vnat[:],
                )
                nc.gpsimd.collective_compute(
                    "AllGather", ALU.bypass, replica_groups=rg,
                    ins=[ag_inV[:]], outs=[ag_outV[:]],
                )

                # Q in DR layout, zero-masked per parity slice:
                # QTz[:, g, v, :] = (ps_g + bq) * mask_v
                QTz = sb.tile([P, HK, 2, EL], FP8, name="QTz")
                for g in range(HK):
                    ps = pmm2.tile([P, 2, EL], F32, tag="mm", name="ps_q")
                    for k in range(HK):
                        nc.tensor.matmul(
                            ps[:, 0, :], inw_sb[:, t, k, g * P:(g + 1) * P],
                            xnT[:, k, :],
                            start=(k == 0), stop=(k == HK - 1),
                        )
                    for v in range(2):
                        nc.scalar.activation(
                            QTz[:, g, v, :], ps[:, 0, :], AF.Identity,
                            scale=mask_sb[:, v:v + 1],
                            bias=inbqm_sb[:, t, g, v:v + 1],
                        )

                # Gather loads: shard slot j -> rotated row block
                kts = [KTl]
                vts = [vnat]
                for j in range(1, NC):
                    row = ((me + j) % NC) * P
                    kt_ = kv.tile([P, HK, EL], FP8, name=f"kt{j}", tag=f"kt{j}")
                    nc.sync.dma_start(
                        kt_[:],
                        ag_outK[bass.ds(row, P), :].rearrange(
                            "p (a b) -> p a b", a=HK
                        ),
                    )
                    kts.append(kt_)
                    vt_ = kv.tile([P, EC, NH, 33], FP8, name=f"vt{j}", tag=f"vt{j}")
                    nc.sync.dma_start(
                        vt_[:],
                        ag_outV[bass.ds(row, P), :].rearrange(
                            "p (a b c) -> p a b c", a=EC, b=NH
                        ),
                    )
                    vts.append(vt_)

                # F. attention. acc bank per head-pair: rows 0:33 / 64:97.
                PAIRS = ((0, 2), (1, 3), (4, 6), (5, 7))
                accs = [
                    pacc.tile([P, EL], F32, tag="acc", name=f"acc{i}")
                    for i in range(4)
                ]
                def qk_pair(ihp, ktile, c):
                    # DoubleRow QK: head h lives in slot s=h//2 (rows
                    # s*32..s*32+32, parity halves), contraction 32 rows x 2
                    # groups, 2 col/cycle; the pair's slots are distinct so
                    # the two streams overlap on row quadrants.
                    hA, hB = PAIRS[ihp]
                    ps2 = pmm2.tile([P, 2, EL], F32, tag="mm", name="ps2")
                    for j, h8 in enumerate((hA, hB)):
                        s = h8 // 2
                        v = h8 % 2
                        rb = s * 32
                        nc.tensor.matmul(
                            ps2[:, j, :],
                            ktile[rb:rb + 32, :, c * P:(c + 1) * P],
                            QTz[rb:rb + 32, :, v, :],
                            start=True, stop=True,
                            perf_mode=PM_DR,
                            tile_position=(rb, 0),
                        )
                    return ps2

                def exp_tile(dst_i8, src, on_act):
                    # Schraudolph: fp8e4 bits = round(s*SCC1 + SCC2)
                    if on_act:
                        nc.scalar.activation(
                            dst_i8, src, AF.Identity, bias=sc2_sb[:],
                            scale=float(SCC1),
                        )
                    else:
                        nc.vector.tensor_scalar(
                            dst_i8, src, float(SCC1), float(SCC2),
                            op0=ALU.mult, op1=ALU.add,
                        )

                def pv_plain(ihp, vt_, c, pt2, start, stop):
                    hA, hB = PAIRS[ihp]
                    for j, h8 in enumerate((hA, hB)):
                        cb = j * 64
                        nc.tensor.matmul(
                            accs[ihp][cb:cb + 33, :],
                            vt_[:, c, h8, :], pt2[:, j, :],
                            start=start, stop=stop,
                            tile_position=(0, cb),
                            skip_group_check=True,
                        )

                # shard 0 (local) first: overlaps the AllGathers.
                # Half-blocks: pairs (blk, blk+2) issue back-to-back so four
                # DR streams cover all four PE row quadrants; the previous
                # half-block's PVs follow (in-order PE never waits on the
                # current exps); exps split ACT/DVE per half-block.
                pending = []
                for j in range(NC):
                    for c in range(EC):
                        for blk in range(2):
                            psA = qk_pair(blk, kts[j], c)
                            psB = qk_pair(blk + 2, kts[j], c)
                            if len(pending) >= 2:
                                pv_plain(*pending.pop(0))
                                pv_plain(*pending.pop(0))
                            ptA = ptp.tile([P, 2, EL], FP8, tag="pt", name="ptA")
                            exp_tile(ptA[:].bitcast(I8), psA[:], True)
                            ptB = ptp.tile([P, 2, EL], FP8, tag="pt", name="ptB")
                            exp_tile(ptB[:].bitcast(I8), psB[:], False)
                            st = j == 0 and c == 0
                            sp = j == NC - 1 and c == EC - 1
                            pending.append((blk, vts[j], c, ptA, st, sp))
                            pending.append((blk + 2, vts[j], c, ptB, st, sp))
                for args in pending:
                    pv_plain(*args)

                # G. normalize: o = num/den per head. Den rows (acc partitions
                # 32/96, 32-aligned) are engine-copied into two staging tiles
                # at aligned slots (even heads -> dstg0, odd -> dstg1, head h
                # at partition (h//2)*32), one reciprocal each, then per-pair
                # selector matmuls broadcast the reciprocals to 32-row bands.
                dstg = [
                    sb.tile([P, EL], F32, name=f"dstg{v}", tag=f"dstg{v}")
                    for v in range(2)
                ]
                for v in range(2):
                    # unused lanes must stay finite: 1/1 = 1, zeroed by bselp
                    nc.vector.memset(dstg[v][:], 1.0)
                for ihp in range(4):
                    for j, h8 in enumerate(PAIRS[ihp]):
                        cb = j * 64
                        nc.scalar.activation(
                            dstg[h8 % 2][(h8 // 2) * 32:(h8 // 2) * 32 + 1, :],
                            accs[ihp][cb + 32:cb + 33, :],
                            AF.Identity, bias=zero_sb[0:1],
                        )
                rec = [sb.tile([P, EL], BF16, name=f"rec{v}", tag=f"rec{v}")
                       for v in range(2)]
                for v in range(2):
                    rcf = sb.tile([P, EL], F32, tag="rcf", name="rcf")
                    nc.vector.reciprocal_approx_fast(rcf[:], dstg[v][:])
                    nc.scalar.activation(
                        rec[v][:], rcf[:], AF.Identity, bias=zero_sb[:]
                    )
                oT = sb.tile([P, HK, EL], BF16, name="oT")
                for ihp in range(4):
                    rcp = pmm2.tile([P, 2, EL], F32, tag="mm", name="rcp")
                    nc.tensor.matmul(
                        rcp[:, 0, :], bselp_sb[:, ihp, :], rec[PAIRS[ihp][0] % 2][:],
                        start=True, stop=True,
                    )
                    rcs = sb.tile([P, EL], F32, tag="rcs", name="rcs")
                    nc.scalar.activation(
                        rcs[:], rcp[:, 0, :], AF.Identity, bias=zero_sb[:]
                    )
                    for j, h8 in enumerate(PAIRS[ihp]):
                        hp = (h8 % 4) * D
                        hf = h8 // 4
                        cb = j * 64
                        nc.vector.tensor_mul(
                            oT[hp:hp + D, hf, :],
                            accs[ihp][cb:cb + 32, :], rcs[cb:cb + 32, :],
                        )

                # H. out-proj + residual: t_ij = attn + 2r (out_b in up_b)
                t_bf = sb.tile([P, EC, H], BF16, name="t_bf")
                for m in range(EC):
                    ps = pmm2.tile([P, 2, EL], F32, tag="mm", name="ps_a")
                    for k in range(HK):
                        nc.tensor.matmul(
                            ps[:, 0, :H], oT[:, k, m * P:(m + 1) * P],
                            outw_sb[:, t, k, :],
                            start=(k == 0), stop=(k == HK - 1),
                        )
                    nc.vector.scalar_tensor_tensor(
                        t_bf[:, m, :], r_nat[:, m, :], 2.0, ps[:, 0, :H],
                        op0=ALU.mult, op1=ALU.add,
                    )

                # I. t^T
                tT = sb.tile([P, HK, EL], BF16, name="tT")
                for c in range(EC):
                    for hf in range(HK):
                        transpose_128(
                            tT[:, hf, c * P:(c + 1) * P],
                            t_bf[:, c, hf * P:(hf + 1) * P],
                        )

                # J. up-proj + LN2 + gelu -> next h (or output)
                last = t == L - 1
                if not last:
                    h_nat_new = sb.tile([P, EC, H], BF16, name="h_nat")
                u4 = sb.tile([P, EC, H], F32, name="u4")
                mv4b = sb.tile([P, EC, 2], F32, name="mv4b")
                for m in range(EC):
                    ps = pmm2.tile([P, 2, EL], F32, tag="mm", name="ps_u")
                    for k in range(HK):
                        nc.tensor.matmul(
                            ps[:, 0, :H], tT[:, k, m * P:(m + 1) * P],
                            upw_sb[:, t, k, :],
                            start=(k == 0), stop=(k == HK - 1),
                        )
                    nc.vector.tensor_add(u4[:, m, :], ps[:, 0, :H], upb2_bc[:, t, :])
                    stats = sb.tile([P, 6], F32, tag="stats", name="stats")
                    nc.vector.bn_stats(stats[:], u4[:, m, :])
                    nc.vector.bn_aggr(mv4b[:, m, :], stats[:])
                rstd4b = sb.tile([P, EC], F32, name="rstd4b")
                nc.scalar.activation(rstd4b[:], mv4b[:, :, 1], AF.Sqrt, bias=eps_sb[:])
                nc.vector.reciprocal(rstd4b[:], rstd4b[:])
                negm4b = sb.tile([P, EC], F32, name="negm4b")
                nc.vector.scalar_tensor_tensor(
                    negm4b[:], mv4b[:, :, 0], -1.0, rstd4b[:],
                    op0=ALU.mult, op1=ALU.mult,
                )
                for m in range(EC):
                    xc = sb.tile([P, H], F32, tag="xln", name="xln")
                    nc.scalar.activation(
                        xc[:], u4[:, m, :], AF.Identity,
                        scale=rstd4b[:, m:m + 1], bias=negm4b[:, m:m + 1],
                    )
                    nc.vector.tensor_mul(xc[:], xc[:], ln2g_bc[:, t, :])
                    uln = sb.tile([P, H], F32, tag="uln", name="uln")
                    nc.vector.tensor_add(uln[:], xc[:], ln2b_bc[:, t, :])
                    if last:
                        hf32 = sb.tile([P, H], F32, tag="hf32", name="hf32")
                        nc.scalar.activation(hf32[:], uln[:], AF.Gelu)
                        nc.sync.dma_start(hout[m * P:(m + 1) * P, :], hf32[:])
                    else:
                        nc.scalar.activation(h_nat_new[:, m, :], uln[:], AF.Gelu)
                if not last:
                    h_nat = h_nat_new

    nc.compile()
    return nc


_NC_CACHE = None


def _get_nc():
    global _NC_CACHE
    if _NC_CACHE is None:
        _NC_CACHE = _build()
    return _NC_CACHE


def _prepare_in_maps(inputs):
    ei = np.asarray(inputs["edge_index"])
    bond = np.asarray(inputs["bond_features"], dtype=np.float32)
    W_emb = np.asarray(inputs["W_emb"], dtype=np.float32)
    b_emb = np.asarray(inputs["b_emb"], dtype=np.float32)
    W_h = np.asarray(inputs["W_h"], dtype=np.float32)
    b_h = np.asarray(inputs["b_h"], dtype=np.float32)
    ln1_g = np.asarray(inputs["ln1_g"], dtype=np.float32)
    ln1_b = np.asarray(inputs["ln1_b"], dtype=np.float32)
    in_w = np.asarray(inputs["in_w"], dtype=np.float32)
    in_b = np.asarray(inputs["in_b"], dtype=np.float32)
    out_w = np.asarray(inputs["out_w"], dtype=np.float32)
    out_b = np.asarray(inputs["out_b"], dtype=np.float32)
    up_w = np.asarray(inputs["up_w"], dtype=np.float32)
    up_b = np.asarray(inputs["up_b"], dtype=np.float32)
    ln2_g = np.asarray(inputs["ln2_g"], dtype=np.float32)
    ln2_b = np.asarray(inputs["ln2_b"], dtype=np.float32)

    tgt = ei[1].astype(np.int64)
    deg = np.zeros(NN, np.float32)
    np.add.at(deg, tgt, 1.0)
    deg_tgt = deg[tgt]  # [E]

    # target-window permutation: core c owns edges with tgt//NW == c
    perm = np.argsort(tgt // NW, kind="stable")
    counts = np.bincount(tgt // NW, minlength=NC * 1)
    counts = np.bincount((tgt // NW).astype(np.int64), minlength=NC)
    assert counts.min() == counts.max() == EL, counts

    # fold LN1 gamma/beta into the in-projection (no 1/sqrt(d) here;
    # it lives in the exp constants / is neutral for V)
    in_w_s = in_w * ln1_g[:, :, None]
    in_b_s = in_b + np.einsum("lh,lho->lo", ln1_b, in_w)

    # DoubleRow column permutation for Q and K blocks: partition
    # p = slot*32 + parity*16 + d in group g holds head 2*slot+parity,
    # dim g*16+d
    px = np.empty(H, np.int64)
    for g in range(HK):
        for p in range(P):
            px[g * P + p] = (2 * (p // 32) + (p % 32) // 16) * 32 \
                + g * 16 + (p % 16)
    in_w_s[:, :, :H] = in_w_s[:, :, px]
    in_w_s[:, :, H:2 * H] = in_w_s[:, :, H + px]
    in_b_s[:, :H] = in_b_s[:, px]
    in_b_s[:, H:2 * H] = 0.0  # K bias cancels under softmax

    shared = {
        "wemb": W_emb.astype(BFNP),
        "bemb": b_emb.reshape(HK, P).T.copy(),
        "wh": W_h.reshape(HK, P, H).transpose(1, 0, 2).astype(BFNP),
        "bh": b_h.reshape(HK, P).T.copy(),
        "inw": in_w_s.reshape(L, HK, P, 3 * H).transpose(2, 0, 1, 3).astype(BFNP),
        "inb": in_b_s.reshape(L, M6, P).transpose(2, 0, 1).copy(),
        "outw": out_w.reshape(L, HK, P, H).transpose(2, 0, 1, 3).astype(BFNP),
        "upw": up_w.reshape(L, HK, P, H).transpose(2, 0, 1, 3).astype(BFNP),
        "upb2": (up_b + np.einsum("lh,lho->lo", out_b, up_w)).astype(np.float32),
        "ln2g": ln2_g, "ln2b": ln2_b,
        "inbv": np.ascontiguousarray(in_b_s[:, 2 * H:3 * H]),
        "qmask": np.ascontiguousarray(
            (((np.arange(P) % 32) // 16)[:, None] == np.arange(2)[None, :]
             ).astype(np.float32)
        ),
    }
    # Q bias premasked: inbqm[p, l, g, v] = in_b_q[l, g*P+p] * qmask[p, v]
    qm = shared["qmask"]
    bq = in_b_s[:, :H].reshape(L, HK, P)
    shared["inbqm"] = np.ascontiguousarray(
        bq.transpose(2, 0, 1)[:, :, :, None] * qm[:, None, None, :]
    )
    # selector for the reciprocal broadcast: head h's den sits at staging
    # partition (h//2)*32; pair's first head covers cols 0:32, second 64:96
    PAIRS_H = ((0, 2), (1, 3), (4, 6), (5, 7))
    bs = np.zeros((P, 4, P), np.float32)
    for ihp, (hA, hB) in enumerate(PAIRS_H):
        bs[(hA // 2) * 32, ihp, 0:32] = 1.0
        bs[(hB // 2) * 32, ihp, 64:96] = 1.0
    shared["bselp"] = np.ascontiguousarray(bs.astype(BFNP))
    shared = {k: np.ascontiguousarray(v) for k, v in shared.items()}

    in_maps = []
    for c in range(NC):
        sl = perm[c * EL:(c + 1) * EL]
        tl = tgt[sl] - c * NW  # local node index in [0, NW)
        dl = deg_tgt[sl]
        A = np.zeros((EL, NW), np.float32)
        A[np.arange(EL), tl] = 1.0
        B = np.zeros((NW, EL), np.float32)
        B[tl, np.arange(EL)] = 1.0
        m = {
            "bondT": np.ascontiguousarray(bond[sl].T.astype(BFNP)),
            "Amat": np.ascontiguousarray(
                A.reshape(EC, P, NW).transpose(1, 0, 2).astype(BFNP)
            ),
            "Bmat": np.ascontiguousarray(
                B.reshape(NW, EC, P).astype(BFNP)
            ),
            "negdeg": np.ascontiguousarray(
                (-dl).reshape(EC, P).T.astype(np.float32)
            ),
        }
        m.update(shared)
        in_maps.append(m)
    return in_maps, perm


def kernel(**inputs):
    nc = _get_nc()
    in_maps, perm = _prepare_in_maps(inputs)
    res = run_bass_kernel_spmd(nc, in_maps, core_ids=list(range(NC)))
    out_perm = np.concatenate(
        [np.asarray(res.results[c]["hout"]) for c in range(NC)], axis=0
    )
    out = np.empty_like(out_perm)
    out[perm] = out_perm
    return out.astype(np.float32)


# revision 45
# speedup vs baseline: 1.1431x; 1.1431x over previous
"""BondMessagePassing kernel for 8 Trainium2 NeuronCores.

Target-window edge sharding: core c owns the 512 edges whose target node
lies in [128c, 128(c+1)), so the segment-sum S and the gather S[tgt] are
fully local (no AllReduce). Per layer:
  - S via one-hot matmul over the local 512 edges -> 128 nodes
  - r = S[tgt] - deg*h fused on DVE
  - full-sequence MHA over 4096 edges in fp8: Q/K/V quantized to fp8e4,
    scores exp'd via the Schraudolph bit trick (round(s*c1+c2) as int8
    bitcast to fp8e4) split across the ACT and DVE engines, probabilities
    consumed by fp8 PV matmuls -- DoubleRow (2 col/cycle) for shard pairs
    (j, j+4), plain fp8 for the local shard (overlapped with the K/V
    AllGathers) and shard 4.
K and V_aug are AllGathered in fp8 as two collectives so QK on remote
shards can start before V lands. The final output is un-permuted on host.
"""

import numpy as np
import ml_dtypes

import concourse.bass as bass
import concourse.tile as tile
import concourse.mybir as mybir
from concourse import bacc
from concourse.bass_utils import run_bass_kernel_spmd
from concourse.masks import make_identity

F32 = mybir.dt.float32
BF16 = mybir.dt.bfloat16
FP8 = mybir.dt.float8e4
I8 = mybir.dt.int8
AF = mybir.ActivationFunctionType
ALU = mybir.AluOpType
BFNP = ml_dtypes.bfloat16
PM_DR = mybir.MatmulPerfMode.DoubleRow

NC = 8          # cores
P = 128         # partitions
NN = 1024       # nodes
E = 4096        # edges
EL = E // NC    # edges per core (512)
H = 256         # hidden
BD = 64         # bond dim
NH = 8          # heads
D = H // NH     # head dim (32)
L = 3           # layers
HK = H // P     # 2  K-chunks per 256
EC = EL // P    # 4  edge chunks per core
M6 = 3 * H // P  # 6 qkv out tiles
NW = NN // NC   # 128 nodes per core (target window)

# Schraudolph exp -> fp8e4 bits: bits = round(s * SCC1 + SCC2)
# (1/sqrt(d) folded into SCC1; tuned offset SCC2)
LOG2E = 1.4426950408889634
SCC1 = 8.0 * LOG2E / float(np.sqrt(np.float32(D)))
SCC2 = 55.7

KB_K = HK * EL            # 1024 fp8 bytes/partition of K^T payload
KB_V = EC * NH * 33       # 1056 fp8 bytes/partition of V_aug payload


def _build():
    nc = bacc.Bacc(None, target_bir_lowering=False, num_devices=NC)

    di = {}
    def din(name, shape, dtype):
        di[name] = nc.dram_tensor(name, shape, dtype, kind="ExternalInput")
        return di[name]

    din("bondT", [BD, EL], BF16)
    din("Amat", [P, EC, NW], BF16)
    din("Bmat", [P, EC, P], BF16)
    din("negdeg", [P, EC], F32)
    din("wemb", [BD, H], BF16)
    din("bemb", [P, HK], F32)
    din("wh", [P, HK, H], BF16)
    din("bh", [P, HK], F32)
    din("inw", [P, L, HK, 3 * H], BF16)
    din("inb", [P, L, M6], F32)
    din("outw", [P, L, HK, H], BF16)
    din("upw", [P, L, HK, H], BF16)
    din("upb2", [L, H], F32)
    din("ln2g", [L, H], F32)
    din("ln2b", [L, H], F32)
    din("inbv", [L, H], F32)
    din("qmask", [P, 2], F32)
    din("inbqm", [P, L, HK, 2], F32)
    din("bselp", [P, 4, P], BF16)
    hout = nc.dram_tensor("hout", [EL, H], F32, kind="ExternalOutput")

    rg = [list(range(NC))]

    with tile.TileContext(nc) as tc:
        with (
            tc.tile_pool(name="const", bufs=1) as const,
            tc.tile_pool(name="sb", bufs=2) as sb,
            tc.tile_pool(name="kv", bufs=1) as kv,
            tc.tile_pool(name="ptp", bufs=6) as ptp,
            tc.tile_pool(name="pmm2", bufs=2, space="PSUM") as pmm2,
            tc.tile_pool(name="pacc", bufs=4, space="PSUM") as pacc,
            tc.tile_pool(name="dram", bufs=1, space="DRAM") as dram,
        ):
            # ---- load constants ----
            bondT_sb = const.tile([BD, EL], BF16)
            nc.sync.dma_start(bondT_sb[:], di["bondT"][:])
            A_sb = const.tile([P, EC, NW], BF16)
            nc.sync.dma_start(A_sb[:], di["Amat"][:])
            B_sb = const.tile([P, EC, P], BF16)
            nc.sync.dma_start(B_sb[:], di["Bmat"][:])
            negdeg_sb = const.tile([P, EC], F32)
            nc.sync.dma_start(negdeg_sb[:], di["negdeg"][:])
            wemb_sb = const.tile([BD, H], BF16)
            nc.sync.dma_start(wemb_sb[:], di["wemb"][:])
            bemb_sb = const.tile([P, HK], F32)
            nc.sync.dma_start(bemb_sb[:], di["bemb"][:])
            wh_sb = const.tile([P, HK, H], BF16)
            nc.sync.dma_start(wh_sb[:], di["wh"][:])
            bh_sb = const.tile([P, HK], F32)
            nc.sync.dma_start(bh_sb[:], di["bh"][:])
            inw_sb = const.tile([P, L, HK, 3 * H], BF16)
            nc.sync.dma_start(inw_sb[:], di["inw"][:])
            inb_sb = const.tile([P, L, M6], F32)
            nc.sync.dma_start(inb_sb[:], di["inb"][:])
            outw_sb = const.tile([P, L, HK, H], BF16)
            nc.sync.dma_start(outw_sb[:], di["outw"][:])
            upw_sb = const.tile([P, L, HK, H], BF16)
            nc.sync.dma_start(upw_sb[:], di["upw"][:])

            def bcast_load(name):
                t = const.tile([P, L, H], F32, name=f"{name}_bc")
                src = di[name][:]
                bap = bass.AP(
                    tensor=src.tensor,
                    offset=src.offset,
                    ap=[[0, P]] + [list(x) for x in src.ap],
                )
                nc.sync.dma_start(t[:], bap)
                return t

            upb2_bc = bcast_load("upb2")
            ln2g_bc = bcast_load("ln2g")
            ln2b_bc = bcast_load("ln2b")
            inbv_bc = bcast_load("inbv")

            ident_bf = const.tile([P, P], BF16)
            make_identity(nc, ident_bf[:])
            eps_sb = const.tile([P, 1], F32)
            nc.vector.memset(eps_sb[:], 1e-5)
            sc2_sb = const.tile([P, 1], F32)
            nc.vector.memset(sc2_sb[:], SCC2)
            zero_sb = const.tile([P, 1], F32)
            nc.vector.memset(zero_sb[:], 0.0)
            # parity masks for the Q zero-slot trick: mask[v][p] = 1 iff
            # (p%32)//16 == v (host-filled)
            mask_sb = const.tile([P, 2], F32)
            nc.sync.dma_start(mask_sb[:], di["qmask"][:])
            inbqm_sb = const.tile([P, L, HK, 2], F32)
            nc.sync.dma_start(inbqm_sb[:], di["inbqm"][:])
            bselp_sb = const.tile([P, 4, P], BF16)
            nc.sync.dma_start(bselp_sb[:], di["bselp"][:])

            def transpose_128(dst_ap, src_ap):
                pst = pmm2.tile([P, P], BF16, tag="mm", name="pst")
                nc.tensor.transpose(pst[:], src_ap, ident_bf[:])
                nc.vector.tensor_copy(dst_ap, pst[:])

            # ---- embedding: h = gelu(bond @ W_emb + b_emb) @ W_h + b_h ----
            g1 = sb.tile([P, HK, EL], BF16, name="g1")
            for m in range(HK):
                ps = pmm2.tile([P, 2, EL], F32, tag="mm", name="ps_e")
                nc.tensor.matmul(
                    ps[:, 0, :], wemb_sb[:, m * P:(m + 1) * P], bondT_sb[:],
                    start=True, stop=True,
                )
                nc.scalar.activation(
                    g1[:, m, :], ps[:, 0, :], AF.Gelu, bias=bemb_sb[:, m:m + 1]
                )
            hT = sb.tile([P, HK, EL], BF16, name="hT")
            for m in range(HK):
                ps = pmm2.tile([P, 2, EL], F32, tag="mm", name="ps_h")
                for k in range(HK):
                    nc.tensor.matmul(
                        ps[:, 0, :], wh_sb[:, k, m * P:(m + 1) * P], g1[:, k, :],
                        start=(k == 0), stop=(k == HK - 1),
                    )
                nc.vector.tensor_scalar_add(hT[:, m, :], ps[:, 0, :], bh_sb[:, m:m + 1])
            h_nat = sb.tile([P, EC, H], BF16, name="h_nat")
            for m in range(HK):
                for c in range(EC):
                    transpose_128(
                        h_nat[:, c, m * P:(m + 1) * P],
                        hT[:, m, c * P:(c + 1) * P],
                    )

            me = nc.sync.partition_id()

            # ---- layers ----
            for t in range(L):
                # A. local segment-sum over the 512 edges targeting this
                # core's 128-node window: S [128 nodes, H]
                psS = pmm2.tile([P, 2, EL], F32, tag="mm", name="psS")
                for c in range(EC):
                    nc.tensor.matmul(
                        psS[:, 0, :H], A_sb[:, c, :], h_nat[:, c, :],
                        start=(c == 0), stop=(c == EC - 1),
                    )
                s_bf = sb.tile([P, H], BF16, name="s_bf")
                nc.scalar.activation(s_bf[:], psS[:, 0, :H], AF.Identity, bias=zero_sb[:])

                # B. r = S[tgt] - deg[tgt]*h: local gather via matmul,
                # diagonal term fused on DVE
                r_nat = sb.tile([P, EC, H], F32, name="r_nat")
                mv4 = sb.tile([P, EC, 2], F32, name="mv4")
                for m in range(EC):
                    ps = pmm2.tile([P, 2, EL], F32, tag="mm", name="ps_r")
                    nc.tensor.matmul(
                        ps[:, 0, :H], B_sb[:, m, :], s_bf[:],
                        start=True, stop=True,
                    )
                    nc.vector.scalar_tensor_tensor(
                        r_nat[:, m, :], h_nat[:, m, :], negdeg_sb[:, m:m + 1],
                        ps[:, 0, :H], op0=ALU.mult, op1=ALU.add,
                    )
                    stats = sb.tile([P, 6], F32, tag="stats", name="stats")
                    nc.vector.bn_stats(stats[:], r_nat[:, m, :])
                    nc.vector.bn_aggr(mv4[:, m, :], stats[:])
                # C. LN1 -> xn (bf16); ln1 gamma/beta folded into in_w/in_b
                rstd4 = sb.tile([P, EC], F32, name="rstd4")
                nc.scalar.activation(rstd4[:], mv4[:, :, 1], AF.Sqrt, bias=eps_sb[:])
                nc.vector.reciprocal(rstd4[:], rstd4[:])
                negm4 = sb.tile([P, EC], F32, name="negm4")
                nc.vector.scalar_tensor_tensor(
                    negm4[:], mv4[:, :, 0], -1.0, rstd4[:],
                    op0=ALU.mult, op1=ALU.mult,
                )
                xn_bf = sb.tile([P, EC, H], BF16, name="xn_bf")
                for m in range(EC):
                    nc.vector.tensor_scalar(
                        xn_bf[:, m, :], r_nat[:, m, :], rstd4[:, m:m + 1],
                        negm4[:, m:m + 1], op0=ALU.mult, op1=ALU.add,
                    )

                # D. xn^T
                xnT = sb.tile([P, HK, EL], BF16, name="xnT")
                for c in range(EC):
                    for hf in range(HK):
                        transpose_128(
                            xnT[:, hf, c * P:(c + 1) * P],
                            xn_bf[:, c, hf * P:(hf + 1) * P],
                        )

                # E. in-proj K first (feeds AG1), then V (AG2), then Q.
                # Q/K in DoubleRow group layout (host-permuted columns):
                # partition p = slot*32 + parity*16 + d holds head 2*slot+
                # parity, dim g*16+d in group slice g. K bias dropped
                # (constant over keys under softmax); Q bias kept.
                KTl = sb.tile([P, HK, EL], FP8, name="KTl")
                for g in range(HK):
                    ps = pmm2.tile([P, 2, EL], F32, tag="mm", name="ps_k")
                    for k in range(HK):
                        nc.tensor.matmul(
                            ps[:, 0, :], inw_sb[:, t, k, (2 + g) * P:(3 + g) * P],
                            xnT[:, k, :],
                            start=(k == 0), stop=(k == HK - 1),
                        )
                    nc.scalar.activation(
                        KTl[:, g, :], ps[:, 0, :], AF.Identity, bias=zero_sb[:]
                    )
                ag_inK = dram.tile([P, KB_K], FP8, name=f"ag_inK{t}")
                ag_outK = dram.tile(
                    [P * NC, KB_K], FP8, addr_space="Shared", name=f"ag_outK{t}"
                )
                nc.sync.dma_start(
                    ag_inK[:].rearrange("p (a b) -> p a b", a=HK), KTl[:]
                )
                nc.gpsimd.collective_compute(
                    "AllGather", ALU.bypass, replica_groups=rg,
                    ins=[ag_inK[:]], outs=[ag_outK[:]],
                )

                # V_aug in fp8: vnat[c, h, 0:32]=v, [...,32]=1.0
                vnat = kv.tile([P, EC, NH, 33], FP8, name="vnat", tag="v0")
                nc.vector.memset(vnat[:, :, :, 32:33], 1.0)
                for c in range(EC):
                    ps = pmm2.tile([P, 2, EL], F32, tag="mm", name="ps_v")
                    for k in range(HK):
                        nc.tensor.matmul(
                            ps[:, 0, :H], xnT[:, k, c * P:(c + 1) * P],
                            inw_sb[:, t, k, 2 * H:3 * H],
                            start=(k == 0), stop=(k == HK - 1),
                        )
                    nc.vector.tensor_add(
                        vnat[:, c, :, 0:32],
                        ps[:, 0, :H].rearrange("p (a b) -> p a b", a=NH),
                        inbv_bc[:, t, :].rearrange("p (a b) -> p a b", a=NH),
                    )
                ag_inV = dram.tile([P, KB_V], FP8, name=f"ag_inV{t}")
                ag_outV = dram.tile(
                    [P * NC, KB_V], FP8, addr_space="Shared", name=f"ag_outV{t}"
                )
                nc.sync.dma_start(
                    ag_inV[:].rearrange("p (a b c) -> p a b c", a=EC, b=NH),
                    vnat[:],
                )
                nc.gpsimd.collective_compute(
                    "AllGather", ALU.bypass, replica_groups=rg,
                    ins=[ag_inV[:]], outs=[ag_outV[:]],
                )

                # Q in DR layout, zero-masked per parity slice:
                # QTz[:, g, v, :] = (ps_g + bq) * mask_v
                QTz = sb.tile([P, HK, 2, EL], FP8, name="QTz")
                for g in range(HK):
                    ps = pmm2.tile([P, 2, EL], F32, tag="mm", name="ps_q")
                    for k in range(HK):
                        nc.tensor.matmul(
                            ps[:, 0, :], inw_sb[:, t, k, g * P:(g + 1) * P],
                            xnT[:, k, :],
                            start=(k == 0), stop=(k == HK - 1),
                        )
                    for v in range(2):
                        nc.vector.tensor_scalar(
                            QTz[:, g, v, :], ps[:, 0, :],
                            mask_sb[:, v:v + 1], inbqm_sb[:, t, g, v:v + 1],
                            op0=ALU.mult, op1=ALU.add,
                        )

                # Gather loads: shard slot j -> rotated row block
                kts = [KTl]
                vts = [vnat]
                for j in range(1, NC):
                    row = ((me + j) % NC) * P
                    kt_ = kv.tile([P, HK, EL], FP8, name=f"kt{j}", tag=f"kt{j}")
                    nc.sync.dma_start(
                        kt_[:],
                        ag_outK[bass.ds(row, P), :].rearrange(
                            "p (a b) -> p a b", a=HK
                        ),
                    )
                    kts.append(kt_)
                    vt_ = kv.tile([P, EC, NH, 33], FP8, name=f"vt{j}", tag=f"vt{j}")
                    nc.sync.dma_start(
                        vt_[:],
                        ag_outV[bass.ds(row, P), :].rearrange(
                            "p (a b c) -> p a b c", a=EC, b=NH
                        ),
                    )
                    vts.append(vt_)

                # F. attention. acc bank per head-pair: rows 0:33 / 64:97.
                PAIRS = ((0, 2), (1, 3), (4, 6), (5, 7))
                accs = [
                    pacc.tile([P, EL], F32, tag="acc", name=f"acc{i}")
                    for i in range(4)
                ]
                def qk_pair(ihp, ktile, c):
                    # DoubleRow QK: head h lives in slot s=h//2 (rows
                    # s*32..s*32+32, parity halves), contraction 32 rows x 2
                    # groups, 2 col/cycle; the pair's slots are distinct so
                    # the two streams overlap on row quadrants.
                    hA, hB = PAIRS[ihp]
                    ps2 = pmm2.tile([P, 2, EL], F32, tag="mm", name="ps2")
                    for j, h8 in enumerate((hA, hB)):
                        s = h8 // 2
                        v = h8 % 2
                        rb = s * 32
                        nc.tensor.matmul(
                            ps2[:, j, :],
                            ktile[rb:rb + 32, :, c * P:(c + 1) * P],
                            QTz[rb:rb + 32, :, v, :],
                            start=True, stop=True,
                            perf_mode=PM_DR,
                            tile_position=(rb, 0),
                        )
                    return ps2

                def exp_tile(dst_i8, src, on_act):
                    # Schraudolph: fp8e4 bits = round(s*SCC1 + SCC2)
                    if on_act:
                        nc.scalar.activation(
                            dst_i8, src, AF.Identity, bias=sc2_sb[:],
                            scale=float(SCC1),
                        )
                    else:
                        nc.vector.tensor_scalar(
                            dst_i8, src, float(SCC1), float(SCC2),
                            op0=ALU.mult, op1=ALU.add,
                        )

                def pv_plain(ihp, vt_, c, pt2, start, stop):
                    hA, hB = PAIRS[ihp]
                    for j, h8 in enumerate((hA, hB)):
                        cb = j * 64
                        nc.tensor.matmul(
                            accs[ihp][cb:cb + 33, :],
                            vt_[:, c, h8, :], pt2[:, j, :],
                            start=start, stop=stop,
                            tile_position=(0, cb),
                            skip_group_check=True,
                        )

                # shard 0 (local) first: overlaps the AllGathers.
                # Half-blocks: pairs (blk, blk+2) issue back-to-back so four
                # DR streams cover all four PE row quadrants; the previous
                # half-block's PVs follow (in-order PE never waits on the
                # current exps); exps split ACT/DVE per half-block.
                pending = []
                for j in range(NC):
                    for c in range(EC):
                        for blk in range(2):
                            psA = qk_pair(blk, kts[j], c)
                            psB = qk_pair(blk + 2, kts[j], c)
                            if len(pending) >= 2:
                                pv_plain(*pending.pop(0))
                                pv_plain(*pending.pop(0))
                            ptA = ptp.tile([P, 2, EL], FP8, tag="pt", name="ptA")
                            exp_tile(ptA[:].bitcast(I8), psA[:], True)
                            ptB = ptp.tile([P, 2, EL], FP8, tag="pt", name="ptB")
                            exp_tile(ptB[:].bitcast(I8), psB[:], False)
                            st = j == 0 and c == 0
                            sp = j == NC - 1 and c == EC - 1
                            pending.append((blk, vts[j], c, ptA, st, sp))
                            pending.append((blk + 2, vts[j], c, ptB, st, sp))
                for args in pending:
                    pv_plain(*args)

                # G. normalize: o = num/den per head. Den rows (acc partitions
                # 32/96, 32-aligned) are engine-copied into two staging tiles
                # at aligned slots (even heads -> dstg0, odd -> dstg1, head h
                # at partition (h//2)*32), one reciprocal each, then per-pair
                # selector matmuls broadcast the reciprocals to 32-row bands.
                dstg = [
                    sb.tile([P, EL], F32, name=f"dstg{v}", tag=f"dstg{v}")
                    for v in range(2)
                ]
                for v in range(2):
                    # unused lanes must stay finite: 1/1 = 1, zeroed by bselp
                    nc.vector.memset(dstg[v][:], 1.0)
                for ihp in range(4):
                    for j, h8 in enumerate(PAIRS[ihp]):
                        cb = j * 64
                        nc.scalar.activation(
                            dstg[h8 % 2][(h8 // 2) * 32:(h8 // 2) * 32 + 1, :],
                            accs[ihp][cb + 32:cb + 33, :],
                            AF.Identity, bias=zero_sb[0:1],
                        )
                rec = [sb.tile([P, EL], BF16, name=f"rec{v}", tag=f"rec{v}")
                       for v in range(2)]
                for v in range(2):
                    rcf = sb.tile([P, EL], F32, tag="rcf", name="rcf")
                    nc.vector.reciprocal_approx_fast(rcf[:], dstg[v][:])
                    nc.scalar.activation(
                        rec[v][:], rcf[:], AF.Identity, bias=zero_sb[:]
                    )
                oT = sb.tile([P, HK, EL], BF16, name="oT")
                for ihp in range(4):
                    rcp = pmm2.tile([P, 2, EL], F32, tag="mm", name="rcp")
                    nc.tensor.matmul(
                        rcp[:, 0, :], bselp_sb[:, ihp, :], rec[PAIRS[ihp][0] % 2][:],
                        start=True, stop=True,
                    )
                    rcs = sb.tile([P, EL], F32, tag="rcs", name="rcs")
                    nc.scalar.activation(
                        rcs[:], rcp[:, 0, :], AF.Identity, bias=zero_sb[:]
                    )
                    for j, h8 in enumerate(PAIRS[ihp]):
                        hp = (h8 % 4) * D
                        hf = h8 // 4
                        cb = j * 64
                        nc.vector.tensor_mul(
                            oT[hp:hp + D, hf, :],
                            accs[ihp][cb:cb + 32, :], rcs[cb:cb + 32, :],
                        )

                # H. out-proj + residual: t_ij = attn + 2r (out_b in up_b)
                t_bf = sb.tile([P, EC, H], BF16, name="t_bf")
                for m in range(EC):
                    ps = pmm2.tile([P, 2, EL], F32, tag="mm", name="ps_a")
                    for k in range(HK):
                        nc.tensor.matmul(
                            ps[:, 0, :H], oT[:, k, m * P:(m + 1) * P],
                            outw_sb[:, t, k, :],
                            start=(k == 0), stop=(k == HK - 1),
                        )
                    nc.vector.scalar_tensor_tensor(
                        t_bf[:, m, :], r_nat[:, m, :], 2.0, ps[:, 0, :H],
                        op0=ALU.mult, op1=ALU.add,
                    )

                # I. t^T
                tT = sb.tile([P, HK, EL], BF16, name="tT")
                for c in range(EC):
                    for hf in range(HK):
                        transpose_128(
                            tT[:, hf, c * P:(c + 1) * P],
                            t_bf[:, c, hf * P:(hf + 1) * P],
                        )

                # J. up-proj + LN2 + gelu -> next h (or output)
                last = t == L - 1
                if not last:
                    h_nat_new = sb.tile([P, EC, H], BF16, name="h_nat")
                u4 = sb.tile([P, EC, H], F32, name="u4")
                mv4b = sb.tile([P, EC, 2], F32, name="mv4b")
                for m in range(EC):
                    ps = pmm2.tile([P, 2, EL], F32, tag="mm", name="ps_u")
                    for k in range(HK):
                        nc.tensor.matmul(
                            ps[:, 0, :H], tT[:, k, m * P:(m + 1) * P],
                            upw_sb[:, t, k, :],
                            start=(k == 0), stop=(k == HK - 1),
                        )
                    nc.vector.tensor_add(u4[:, m, :], ps[:, 0, :H], upb2_bc[:, t, :])
                    stats = sb.tile([P, 6], F32, tag="stats", name="stats")
                    nc.vector.bn_stats(stats[:], u4[:, m, :])
                    nc.vector.bn_aggr(mv4b[:, m, :], stats[:])
                rstd4b = sb.tile([P, EC], F32, name="rstd4b")
                nc.scalar.activation(rstd4b[:], mv4b[:, :, 1], AF.Sqrt, bias=eps_sb[:])
                nc.vector.reciprocal(rstd4b[:], rstd4b[:])
                negm4b = sb.tile([P, EC], F32, name="negm4b")
                nc.vector.scalar_tensor_tensor(
                    negm4b[:], mv4b[:, :, 0], -1.0, rstd4b[:],
                    op0=ALU.mult, op1=ALU.mult,
                )
                for m in range(EC):
                    xc = sb.tile([P, H], F32, tag="xln", name="xln")
                    nc.vector.tensor_scalar(
                        xc[:], u4[:, m, :], rstd4b[:, m:m + 1],
                        negm4b[:, m:m + 1], op0=ALU.mult, op1=ALU.add,
                    )
                    nc.vector.tensor_mul(xc[:], xc[:], ln2g_bc[:, t, :])
                    uln = sb.tile([P, H], F32, tag="uln", name="uln")
                    nc.vector.tensor_add(uln[:], xc[:], ln2b_bc[:, t, :])
                    if last:
                        hf32 = sb.tile([P, H], F32, tag="hf32", name="hf32")
                        nc.scalar.activation(hf32[:], uln[:], AF.Gelu)
                        nc.sync.dma_start(hout[m * P:(m + 1) * P, :], hf32[:])
                    else:
                        nc.scalar.activation(h_nat_new[:, m, :], uln[:], AF.Gelu)
                if not last:
                    h_nat = h_nat_new

    nc.compile()
    return nc


_NC_CACHE = None


def _get_nc():
    global _NC_CACHE
    if _NC_CACHE is None:
        _NC_CACHE = _build()
    return _NC_CACHE


def _prepare_in_maps(inputs):
    ei = np.asarray(inputs["edge_index"])
    bond = np.asarray(inputs["bond_features"], dtype=np.float32)
    W_emb = np.asarray(inputs["W_emb"], dtype=np.float32)
    b_emb = np.asarray(inputs["b_emb"], dtype=np.float32)
    W_h = np.asarray(inputs["W_h"], dtype=np.float32)
    b_h = np.asarray(inputs["b_h"], dtype=np.float32)
    ln1_g = np.asarray(inputs["ln1_g"], dtype=np.float32)
    ln1_b = np.asarray(inputs["ln1_b"], dtype=np.float32)
    in_w = np.asarray(inputs["in_w"], dtype=np.float32)
    in_b = np.asarray(inputs["in_b"], dtype=np.float32)
    out_w = np.asarray(inputs["out_w"], dtype=np.float32)
    out_b = np.asarray(inputs["out_b"], dtype=np.float32)
    up_w = np.asarray(inputs["up_w"], dtype=np.float32)
    up_b = np.asarray(inputs["up_b"], dtype=np.float32)
    ln2_g = np.asarray(inputs["ln2_g"], dtype=np.float32)
    ln2_b = np.asarray(inputs["ln2_b"], dtype=np.float32)

    tgt = ei[1].astype(np.int64)
    deg = np.zeros(NN, np.float32)
    np.add.at(deg, tgt, 1.0)
    deg_tgt = deg[tgt]  # [E]

    # target-window permutation: core c owns edges with tgt//NW == c
    perm = np.argsort(tgt // NW, kind="stable")
    counts = np.bincount(tgt // NW, minlength=NC * 1)
    counts = np.bincount((tgt // NW).astype(np.int64), minlength=NC)
    assert counts.min() == counts.max() == EL, counts

    # fold LN1 gamma/beta into the in-projection (no 1/sqrt(d) here;
    # it lives in the exp constants / is neutral for V)
    in_w_s = in_w * ln1_g[:, :, None]
    in_b_s = in_b + np.einsum("lh,lho->lo", ln1_b, in_w)

    # DoubleRow column permutation for Q and K blocks: partition
    # p = slot*32 + parity*16 + d in group g holds head 2*slot+parity,
    # dim g*16+d
    px = np.empty(H, np.int64)
    for g in range(HK):
        for p in range(P):
            px[g * P + p] = (2 * (p // 32) + (p % 32) // 16) * 32 \
                + g * 16 + (p % 16)
    in_w_s[:, :, :H] = in_w_s[:, :, px]
    in_w_s[:, :, H:2 * H] = in_w_s[:, :, H + px]
    in_b_s[:, :H] = in_b_s[:, px]
    in_b_s[:, H:2 * H] = 0.0  # K bias cancels under softmax

    shared = {
        "wemb": W_emb.astype(BFNP),
        "bemb": b_emb.reshape(HK, P).T.copy(),
        "wh": W_h.reshape(HK, P, H).transpose(1, 0, 2).astype(BFNP),
        "bh": b_h.reshape(HK, P).T.copy(),
        "inw": in_w_s.reshape(L, HK, P, 3 * H).transpose(2, 0, 1, 3).astype(BFNP),
        "inb": in_b_s.reshape(L, M6, P).transpose(2, 0, 1).copy(),
        "outw": out_w.reshape(L, HK, P, H).transpose(2, 0, 1, 3).astype(BFNP),
        "upw": up_w.reshape(L, HK, P, H).transpose(2, 0, 1, 3).astype(BFNP),
        "upb2": (up_b + np.einsum("lh,lho->lo", out_b, up_w)).astype(np.float32),
        "ln2g": ln2_g, "ln2b": ln2_b,
        "inbv": np.ascontiguousarray(in_b_s[:, 2 * H:3 * H]),
        "qmask": np.ascontiguousarray(
            (((np.arange(P) % 32) // 16)[:, None] == np.arange(2)[None, :]
             ).astype(np.float32)
        ),
    }
    # Q bias premasked: inbqm[p, l, g, v] = in_b_q[l, g*P+p] * qmask[p, v]
    qm = shared["qmask"]
    bq = in_b_s[:, :H].reshape(L, HK, P)
    shared["inbqm"] = np.ascontiguousarray(
        bq.transpose(2, 0, 1)[:, :, :, None] * qm[:, None, None, :]
    )
    # selector for the reciprocal broadcast: head h's den sits at staging
    # partition (h//2)*32; pair's first head covers cols 0:32, second 64:96
    PAIRS_H = ((0, 2), (1, 3), (4, 6), (5, 7))
    bs = np.zeros((P, 4, P), np.float32)
    for ihp, (hA, hB) in enumerate(PAIRS_H):
        bs[(hA // 2) * 32, ihp, 0:32] = 1.0
        bs[(hB // 2) * 32, ihp, 64:96] = 1.0
    shared["bselp"] = np.ascontiguousarray(bs.astype(BFNP))
    shared = {k: np.ascontiguousarray(v) for k, v in shared.items()}

    in_maps = []
    for c in range(NC):
        sl = perm[c * EL:(c + 1) * EL]
        tl = tgt[sl] - c * NW  # local node index in [0, NW)
        dl = deg_tgt[sl]
        A = np.zeros((EL, NW), np.float32)
        A[np.arange(EL), tl] = 1.0
        B = np.zeros((NW, EL), np.float32)
        B[tl, np.arange(EL)] = 1.0
        m = {
            "bondT": np.ascontiguousarray(bond[sl].T.astype(BFNP)),
            "Amat": np.ascontiguousarray(
                A.reshape(EC, P, NW).transpose(1, 0, 2).astype(BFNP)
            ),
            "Bmat": np.ascontiguousarray(
                B.reshape(NW, EC, P).astype(BFNP)
            ),
            "negdeg": np.ascontiguousarray(
                (-dl).reshape(EC, P).T.astype(np.float32)
            ),
        }
        m.update(shared)
        in_maps.append(m)
    return in_maps, perm


def kernel(**inputs):
    nc = _get_nc()
    in_maps, perm = _prepare_in_maps(inputs)
    res = run_bass_kernel_spmd(nc, in_maps, core_ids=list(range(NC)))
    out_perm = np.concatenate(
        [np.asarray(res.results[c]["hout"]) for c in range(NC)], axis=0
    )
    out = np.empty_like(out_perm)
    out[perm] = out_perm
    return out.astype(np.float32)


# revision 53
# speedup vs baseline: 1.2565x; 1.0992x over previous
"""BondMessagePassing kernel for 8 Trainium2 NeuronCores.

Target-window edge sharding: core c owns the 512 edges whose target node
lies in [128c, 128(c+1)), so the segment-sum S and the gather S[tgt] are
fully local (no AllReduce). Per layer:
  - S via one-hot matmul over the local 512 edges -> 128 nodes
  - r = S[tgt] - deg*h fused on DVE
  - full-sequence MHA over 4096 edges in fp8: Q/K/V quantized to fp8e4,
    scores exp'd via the Schraudolph bit trick (round(s*c1+c2) as int8
    bitcast to fp8e4) split across the ACT and DVE engines, probabilities
    consumed by fp8 PV matmuls -- DoubleRow (2 col/cycle) for shard pairs
    (j, j+4), plain fp8 for the local shard (overlapped with the K/V
    AllGathers) and shard 4.
K and V_aug are AllGathered in fp8 as two collectives so QK on remote
shards can start before V lands. The final output is un-permuted on host.
"""

import numpy as np
import ml_dtypes

import concourse.bass as bass
import concourse.tile as tile
import concourse.mybir as mybir
from concourse import bacc
from concourse.bass_utils import run_bass_kernel_spmd
from concourse.masks import make_identity

F32 = mybir.dt.float32
BF16 = mybir.dt.bfloat16
FP8 = mybir.dt.float8e4
I8 = mybir.dt.int8
AF = mybir.ActivationFunctionType
ALU = mybir.AluOpType
BFNP = ml_dtypes.bfloat16
PM_DR = mybir.MatmulPerfMode.DoubleRow

NC = 8          # cores
P = 128         # partitions
NN = 1024       # nodes
E = 4096        # edges
EL = E // NC    # edges per core (512)
H = 256         # hidden
BD = 64         # bond dim
NH = 8          # heads
D = H // NH     # head dim (32)
L = 3           # layers
HK = H // P     # 2  K-chunks per 256
EC = EL // P    # 4  edge chunks per core
M6 = 3 * H // P  # 6 qkv out tiles
NW = NN // NC   # 128 nodes per core (target window)

# Schraudolph exp -> fp8e4 bits: bits = round(s * SCC1 + SCC2)
# (1/sqrt(d) folded into SCC1; tuned offset SCC2)
LOG2E = 1.4426950408889634
SCC1 = 8.0 * LOG2E / float(np.sqrt(np.float32(D)))
SCC2 = 55.7

KB_K = HK * EL            # 1024 fp8 bytes/partition of K^T payload
KB_V = EC * NH * 33       # 1056 fp8 bytes/partition of V_aug payload


def _build():
    nc = bacc.Bacc(None, target_bir_lowering=False, num_devices=NC)

    di = {}
    def din(name, shape, dtype):
        di[name] = nc.dram_tensor(name, shape, dtype, kind="ExternalInput")
        return di[name]

    din("bondT", [BD, EL], BF16)
    din("Amat", [P, EC, NW], BF16)
    din("Bmat", [P, EC, P], BF16)
    din("negdeg", [P, EC], F32)
    din("wemb", [BD, H], BF16)
    din("bemb", [P, HK], F32)
    din("wh", [P, HK, H], BF16)
    din("bh", [P, HK], F32)
    din("inw", [P, L, HK, 3 * H], BF16)
    din("inb", [P, L, M6], F32)
    din("outw", [P, L, HK, H], BF16)
    din("upw", [P, L, HK, H], BF16)
    din("upb2", [L, H], F32)
    din("ln2g", [L, H], F32)
    din("ln2b", [L, H], F32)
    din("inbv", [L, H], F32)
    din("bselp", [P, 4, P], BF16)
    hout = nc.dram_tensor("hout", [EL, H], F32, kind="ExternalOutput")

    rg = [list(range(NC))]

    with tile.TileContext(nc) as tc:
        with (
            tc.tile_pool(name="const", bufs=1) as const,
            tc.tile_pool(name="sb", bufs=2) as sb,
            tc.tile_pool(name="kv", bufs=1) as kv,
            tc.tile_pool(name="ptp", bufs=6) as ptp,
            tc.tile_pool(name="pmm2", bufs=2, space="PSUM") as pmm2,
            tc.tile_pool(name="pacc", bufs=4, space="PSUM") as pacc,
            tc.tile_pool(name="dram", bufs=1, space="DRAM") as dram,
        ):
            # ---- load constants ----
            bondT_sb = const.tile([BD, EL], BF16)
            nc.sync.dma_start(bondT_sb[:], di["bondT"][:])
            A_sb = const.tile([P, EC, NW], BF16)
            nc.sync.dma_start(A_sb[:], di["Amat"][:])
            B_sb = const.tile([P, EC, P], BF16)
            nc.sync.dma_start(B_sb[:], di["Bmat"][:])
            negdeg_sb = const.tile([P, EC], F32)
            nc.sync.dma_start(negdeg_sb[:], di["negdeg"][:])
            wemb_sb = const.tile([BD, H], BF16)
            nc.sync.dma_start(wemb_sb[:], di["wemb"][:])
            bemb_sb = const.tile([P, HK], F32)
            nc.sync.dma_start(bemb_sb[:], di["bemb"][:])
            wh_sb = const.tile([P, HK, H], BF16)
            nc.sync.dma_start(wh_sb[:], di["wh"][:])
            bh_sb = const.tile([P, HK], F32)
            nc.sync.dma_start(bh_sb[:], di["bh"][:])
            inw_sb = const.tile([P, L, HK, 3 * H], BF16)
            nc.sync.dma_start(inw_sb[:], di["inw"][:])
            inb_sb = const.tile([P, L, M6], F32)
            nc.sync.dma_start(inb_sb[:], di["inb"][:])
            outw_sb = const.tile([P, L, HK, H], BF16)
            nc.sync.dma_start(outw_sb[:], di["outw"][:])
            upw_sb = const.tile([P, L, HK, H], BF16)
            nc.sync.dma_start(upw_sb[:], di["upw"][:])

            def bcast_load(name):
                t = const.tile([P, L, H], F32, name=f"{name}_bc")
                src = di[name][:]
                bap = bass.AP(
                    tensor=src.tensor,
                    offset=src.offset,
                    ap=[[0, P]] + [list(x) for x in src.ap],
                )
                nc.sync.dma_start(t[:], bap)
                return t

            upb2_bc = bcast_load("upb2")
            ln2g_bc = bcast_load("ln2g")
            ln2b_bc = bcast_load("ln2b")
            inbv_bc = bcast_load("inbv")

            ident_bf = const.tile([P, P], BF16)
            make_identity(nc, ident_bf[:])
            eps_sb = const.tile([P, 1], F32)
            nc.vector.memset(eps_sb[:], 1e-5)
            sc2_sb = const.tile([P, 1], F32)
            nc.vector.memset(sc2_sb[:], SCC2)
            zero_sb = const.tile([P, 1], F32)
            nc.vector.memset(zero_sb[:], 0.0)
            bselp_sb = const.tile([P, 4, P], BF16)
            nc.sync.dma_start(bselp_sb[:], di["bselp"][:])

            def transpose_128(dst_ap, src_ap):
                pst = pmm2.tile([P, P], BF16, tag="mm", name="pst")
                nc.tensor.transpose(pst[:], src_ap, ident_bf[:])
                nc.vector.tensor_copy(dst_ap, pst[:])

            # ---- embedding: h = gelu(bond @ W_emb + b_emb) @ W_h + b_h ----
            g1 = sb.tile([P, HK, EL], BF16, name="g1")
            for m in range(HK):
                ps = pmm2.tile([P, 2, EL], F32, tag="mm", name="ps_e")
                nc.tensor.matmul(
                    ps[:, 0, :], wemb_sb[:, m * P:(m + 1) * P], bondT_sb[:],
                    start=True, stop=True,
                )
                nc.scalar.activation(
                    g1[:, m, :], ps[:, 0, :], AF.Gelu, bias=bemb_sb[:, m:m + 1]
                )
            hT = sb.tile([P, HK, EL], BF16, name="hT")
            for m in range(HK):
                ps = pmm2.tile([P, 2, EL], F32, tag="mm", name="ps_h")
                for k in range(HK):
                    nc.tensor.matmul(
                        ps[:, 0, :], wh_sb[:, k, m * P:(m + 1) * P], g1[:, k, :],
                        start=(k == 0), stop=(k == HK - 1),
                    )
                nc.vector.tensor_scalar_add(hT[:, m, :], ps[:, 0, :], bh_sb[:, m:m + 1])
            h_nat = sb.tile([P, EC, H], BF16, name="h_nat")
            for m in range(HK):
                for c in range(EC):
                    transpose_128(
                        h_nat[:, c, m * P:(m + 1) * P],
                        hT[:, m, c * P:(c + 1) * P],
                    )

            me = nc.sync.partition_id()

            # ---- layers ----
            for t in range(L):
                # A. local segment-sum over the 512 edges targeting this
                # core's 128-node window: S [128 nodes, H]
                psS = pmm2.tile([P, 2, EL], F32, tag="mm", name="psS")
                for c in range(EC):
                    nc.tensor.matmul(
                        psS[:, 0, :H], A_sb[:, c, :], h_nat[:, c, :],
                        start=(c == 0), stop=(c == EC - 1),
                    )
                s_bf = sb.tile([P, H], BF16, name="s_bf")
                nc.scalar.activation(s_bf[:], psS[:, 0, :H], AF.Identity, bias=zero_sb[:])

                # B. r = S[tgt] - deg[tgt]*h: local gather via matmul,
                # diagonal term fused on DVE
                r_nat = sb.tile([P, EC, H], F32, name="r_nat")
                mv4 = sb.tile([P, EC, 2], F32, name="mv4")
                for m in range(EC):
                    ps = pmm2.tile([P, 2, EL], F32, tag="mm", name="ps_r")
                    nc.tensor.matmul(
                        ps[:, 0, :H], B_sb[:, m, :], s_bf[:],
                        start=True, stop=True,
                    )
                    nc.vector.scalar_tensor_tensor(
                        r_nat[:, m, :], h_nat[:, m, :], negdeg_sb[:, m:m + 1],
                        ps[:, 0, :H], op0=ALU.mult, op1=ALU.add,
                    )
                    stats = sb.tile([P, 6], F32, tag="stats", name="stats")
                    nc.vector.bn_stats(stats[:], r_nat[:, m, :])
                    nc.vector.bn_aggr(mv4[:, m, :], stats[:])
                # C. LN1 -> xn (bf16); ln1 gamma/beta folded into in_w/in_b
                rstd4 = sb.tile([P, EC], F32, name="rstd4")
                nc.scalar.activation(rstd4[:], mv4[:, :, 1], AF.Sqrt, bias=eps_sb[:])
                nc.vector.reciprocal(rstd4[:], rstd4[:])
                negm4 = sb.tile([P, EC], F32, name="negm4")
                nc.vector.scalar_tensor_tensor(
                    negm4[:], mv4[:, :, 0], -1.0, rstd4[:],
                    op0=ALU.mult, op1=ALU.mult,
                )
                xn_bf = sb.tile([P, EC, H], BF16, name="xn_bf")
                for m in range(EC):
                    nc.vector.tensor_scalar(
                        xn_bf[:, m, :], r_nat[:, m, :], rstd4[:, m:m + 1],
                        negm4[:, m:m + 1], op0=ALU.mult, op1=ALU.add,
                    )

                # D. xn^T
                xnT = sb.tile([P, HK, EL], BF16, name="xnT")
                for c in range(EC):
                    for hf in range(HK):
                        transpose_128(
                            xnT[:, hf, c * P:(c + 1) * P],
                            xn_bf[:, c, hf * P:(hf + 1) * P],
                        )

                # E. in-proj K first (feeds AG1), then V (AG2), then Q.
                # Head h occupies rows (h%4)*32, column chunk h//4. K bias
                # dropped (constant over keys under softmax); Q bias kept.
                KTl = sb.tile([P, HK, EL], FP8, name="KTl")
                for g in range(HK):
                    ps = pmm2.tile([P, 2, EL], F32, tag="mm", name="ps_k")
                    for k in range(HK):
                        nc.tensor.matmul(
                            ps[:, 0, :], inw_sb[:, t, k, (2 + g) * P:(3 + g) * P],
                            xnT[:, k, :],
                            start=(k == 0), stop=(k == HK - 1),
                        )
                    nc.scalar.activation(
                        KTl[:, g, :], ps[:, 0, :], AF.Identity, bias=zero_sb[:]
                    )
                ag_inK = dram.tile([P, KB_K], FP8, name=f"ag_inK{t}")
                ag_outK = dram.tile(
                    [P * NC, KB_K], FP8, addr_space="Shared", name=f"ag_outK{t}"
                )
                nc.sync.dma_start(
                    ag_inK[:].rearrange("p (a b) -> p a b", a=HK), KTl[:]
                )
                nc.gpsimd.collective_compute(
                    "AllGather", ALU.bypass, replica_groups=rg,
                    ins=[ag_inK[:]], outs=[ag_outK[:]],
                )

                # V_aug in fp8: vnat[c, h, 0:32]=v, [...,32]=1.0
                vnat = kv.tile([P, EC, NH, 33], FP8, name="vnat", tag="v0")
                nc.vector.memset(vnat[:, :, :, 32:33], 1.0)
                for c in range(EC):
                    ps = pmm2.tile([P, 2, EL], F32, tag="mm", name="ps_v")
                    for k in range(HK):
                        nc.tensor.matmul(
                            ps[:, 0, :H], xnT[:, k, c * P:(c + 1) * P],
                            inw_sb[:, t, k, 2 * H:3 * H],
                            start=(k == 0), stop=(k == HK - 1),
                        )
                    nc.vector.tensor_add(
                        vnat[:, c, :, 0:32],
                        ps[:, 0, :H].rearrange("p (a b) -> p a b", a=NH),
                        inbv_bc[:, t, :].rearrange("p (a b) -> p a b", a=NH),
                    )
                ag_inV = dram.tile([P, KB_V], FP8, name=f"ag_inV{t}")
                ag_outV = dram.tile(
                    [P * NC, KB_V], FP8, addr_space="Shared", name=f"ag_outV{t}"
                )
                nc.sync.dma_start(
                    ag_inV[:].rearrange("p (a b c) -> p a b c", a=EC, b=NH),
                    vnat[:],
                )
                nc.gpsimd.collective_compute(
                    "AllGather", ALU.bypass, replica_groups=rg,
                    ins=[ag_inV[:]], outs=[ag_outV[:]],
                )

                # Q with its bias (32-row tiles need no zero padding)
                QTz = sb.tile([P, HK, EL], FP8, name="QTz")
                for g in range(HK):
                    ps = pmm2.tile([P, 2, EL], F32, tag="mm", name="ps_q")
                    for k in range(HK):
                        nc.tensor.matmul(
                            ps[:, 0, :], inw_sb[:, t, k, g * P:(g + 1) * P],
                            xnT[:, k, :],
                            start=(k == 0), stop=(k == HK - 1),
                        )
                    nc.vector.tensor_scalar_add(
                        QTz[:, g, :], ps[:, 0, :], inb_sb[:, t, g:g + 1]
                    )

                # Gather loads: shard slot j -> rotated row block
                kts = [KTl]
                vts = [vnat]
                for j in range(1, NC):
                    row = ((me + j) % NC) * P
                    kt_ = kv.tile([P, HK, EL], FP8, name=f"kt{j}", tag=f"kt{j}")
                    nc.sync.dma_start(
                        kt_[:],
                        ag_outK[bass.ds(row, P), :].rearrange(
                            "p (a b) -> p a b", a=HK
                        ),
                    )
                    kts.append(kt_)
                    vt_ = kv.tile([P, EC, NH, 33], FP8, name=f"vt{j}", tag=f"vt{j}")
                    nc.sync.dma_start(
                        vt_[:],
                        ag_outV[bass.ds(row, P), :].rearrange(
                            "p (a b c) -> p a b c", a=EC, b=NH
                        ),
                    )
                    vts.append(vt_)

                # F. attention. acc bank per head-pair: rows 0:33 / 64:97.
                PAIRS = ((0, 2), (1, 3), (4, 6), (5, 7))
                accs = [
                    pacc.tile([P, EL], F32, tag="acc", name=f"acc{i}")
                    for i in range(4)
                ]
                def qk_pair(ihp, ktile, c):
                    # plain fp8 QK, 32-row tiles: head h at rows (h%4)*32,
                    # col chunk h//4; the pair's rows are disjoint so the two
                    # streams overlap, and the half-block partner pair covers
                    # the other two 32-row quadrants.
                    hA, hB = PAIRS[ihp]
                    ps2 = pmm2.tile([P, 2, EL], F32, tag="mm", name="ps2")
                    for j, h8 in enumerate((hA, hB)):
                        rb = (h8 % 4) * 32
                        hf = h8 // 4
                        nc.tensor.matmul(
                            ps2[:, j, :],
                            ktile[rb:rb + 32, hf, c * P:(c + 1) * P],
                            QTz[rb:rb + 32, hf, :],
                            start=True, stop=True,
                            tile_position=(rb, 0),
                        )
                    return ps2

                def exp_tile(dst_i8, src, on_act):
                    # Schraudolph: fp8e4 bits = round(s*SCC1 + SCC2)
                    if on_act:
                        nc.scalar.activation(
                            dst_i8, src, AF.Identity, bias=sc2_sb[:],
                            scale=float(SCC1),
                        )
                    else:
                        nc.vector.tensor_scalar(
                            dst_i8, src, float(SCC1), float(SCC2),
                            op0=ALU.mult, op1=ALU.add,
                        )

                def pv_plain(ihp, vt_, c, pt2, start, stop):
                    hA, hB = PAIRS[ihp]
                    for j, h8 in enumerate((hA, hB)):
                        cb = j * 64
                        nc.tensor.matmul(
                            accs[ihp][cb:cb + 33, :],
                            vt_[:, c, h8, :], pt2[:, j, :],
                            start=start, stop=stop,
                            tile_position=(0, cb),
                            skip_group_check=True,
                        )

                # shard 0 (local) first: overlaps the AllGathers.
                # Half-blocks: pairs (blk, blk+2) issue back-to-back so four
                # DR streams cover all four PE row quadrants; the previous
                # half-block's PVs follow (in-order PE never waits on the
                # current exps); exps split ACT/DVE per half-block.
                pending = []
                for j in range(NC):
                    for c in range(EC):
                        for blk in range(2):
                            # pairs (blk, 3-blk): rows {0,64} + {32,96} ->
                            # all four 32-row quadrants in flight
                            pA, pB = blk, 3 - blk
                            psA = qk_pair(pA, kts[j], c)
                            psB = qk_pair(pB, kts[j], c)
                            if len(pending) >= 2:
                                pv_plain(*pending.pop(0))
                                pv_plain(*pending.pop(0))
                            # DVE (slower exp) takes the earlier tile
                            ptA = ptp.tile([P, 2, EL], FP8, tag="pt", name="ptA")
                            exp_tile(ptA[:].bitcast(I8), psA[:], False)
                            ptB = ptp.tile([P, 2, EL], FP8, tag="pt", name="ptB")
                            exp_tile(ptB[:].bitcast(I8), psB[:], True)
                            st = j == 0 and c == 0
                            sp = j == NC - 1 and c == EC - 1
                            pending.append((pA, vts[j], c, ptA, st, sp))
                            pending.append((pB, vts[j], c, ptB, st, sp))
                for args in pending:
                    pv_plain(*args)

                # G. normalize: o = num/den per head. Den rows (acc partitions
                # 32/96, 32-aligned) are engine-copied into two staging tiles
                # at aligned slots (even heads -> dstg0, odd -> dstg1, head h
                # at partition (h//2)*32), one reciprocal each, then per-pair
                # selector matmuls broadcast the reciprocals to 32-row bands.
                dstg = [
                    sb.tile([P, EL], F32, name=f"dstg{v}", tag=f"dstg{v}")
                    for v in range(2)
                ]
                for v in range(2):
                    # unused lanes must stay finite: 1/1 = 1, zeroed by bselp
                    nc.vector.memset(dstg[v][:], 1.0)
                for ihp in range(4):
                    for j, h8 in enumerate(PAIRS[ihp]):
                        cb = j * 64
                        nc.scalar.activation(
                            dstg[h8 % 2][(h8 // 2) * 32:(h8 // 2) * 32 + 1, :],
                            accs[ihp][cb + 32:cb + 33, :],
                            AF.Identity, bias=zero_sb[0:1],
                        )
                rec = [sb.tile([P, EL], BF16, name=f"rec{v}", tag=f"rec{v}")
                       for v in range(2)]
                for v in range(2):
                    rcf = sb.tile([P, EL], F32, tag="rcf", name="rcf")
                    nc.vector.reciprocal_approx_fast(rcf[:], dstg[v][:])
                    nc.scalar.activation(
                        rec[v][:], rcf[:], AF.Identity, bias=zero_sb[:]
                    )
                oT = sb.tile([P, HK, EL], BF16, name="oT")
                for ihp in range(4):
                    rcp = pmm2.tile([P, 2, EL], F32, tag="mm", name="rcp")
                    nc.tensor.matmul(
                        rcp[:, 0, :], bselp_sb[:, ihp, :], rec[PAIRS[ihp][0] % 2][:],
                        start=True, stop=True,
                    )
                    rcs = sb.tile([P, EL], F32, tag="rcs", name="rcs")
                    nc.scalar.activation(
                        rcs[:], rcp[:, 0, :], AF.Identity, bias=zero_sb[:]
                    )
                    for j, h8 in enumerate(PAIRS[ihp]):
                        hp = (h8 % 4) * D
                        hf = h8 // 4
                        cb = j * 64
                        nc.vector.tensor_mul(
                            oT[hp:hp + D, hf, :],
                            accs[ihp][cb:cb + 32, :], rcs[cb:cb + 32, :],
                        )

                # H. out-proj + residual: t_ij = attn + 2r (out_b in up_b)
                t_bf = sb.tile([P, EC, H], BF16, name="t_bf")
                for m in range(EC):
                    ps = pmm2.tile([P, 2, EL], F32, tag="mm", name="ps_a")
                    for k in range(HK):
                        nc.tensor.matmul(
                            ps[:, 0, :H], oT[:, k, m * P:(m + 1) * P],
                            outw_sb[:, t, k, :],
                            start=(k == 0), stop=(k == HK - 1),
                        )
                    nc.vector.scalar_tensor_tensor(
                        t_bf[:, m, :], r_nat[:, m, :], 2.0, ps[:, 0, :H],
                        op0=ALU.mult, op1=ALU.add,
                    )

                # I. t^T
                tT = sb.tile([P, HK, EL], BF16, name="tT")
                for c in range(EC):
                    for hf in range(HK):
                        transpose_128(
                            tT[:, hf, c * P:(c + 1) * P],
                            t_bf[:, c, hf * P:(hf + 1) * P],
                        )

                # J. up-proj + LN2 + gelu -> next h (or output)
                last = t == L - 1
                if not last:
                    h_nat_new = sb.tile([P, EC, H], BF16, name="h_nat")
                u4 = sb.tile([P, EC, H], F32, name="u4")
                mv4b = sb.tile([P, EC, 2], F32, name="mv4b")
                for m in range(EC):
                    ps = pmm2.tile([P, 2, EL], F32, tag="mm", name="ps_u")
                    for k in range(HK):
                        nc.tensor.matmul(
                            ps[:, 0, :H], tT[:, k, m * P:(m + 1) * P],
                            upw_sb[:, t, k, :],
                            start=(k == 0), stop=(k == HK - 1),
                        )
                    nc.vector.tensor_add(u4[:, m, :], ps[:, 0, :H], upb2_bc[:, t, :])
                    stats = sb.tile([P, 6], F32, tag="stats", name="stats")
                    nc.vector.bn_stats(stats[:], u4[:, m, :])
                    nc.vector.bn_aggr(mv4b[:, m, :], stats[:])
                rstd4b = sb.tile([P, EC], F32, name="rstd4b")
                nc.scalar.activation(rstd4b[:], mv4b[:, :, 1], AF.Sqrt, bias=eps_sb[:])
                nc.vector.reciprocal(rstd4b[:], rstd4b[:])
                negm4b = sb.tile([P, EC], F32, name="negm4b")
                nc.vector.scalar_tensor_tensor(
                    negm4b[:], mv4b[:, :, 0], -1.0, rstd4b[:],
                    op0=ALU.mult, op1=ALU.mult,
                )
                for m in range(EC):
                    xc = sb.tile([P, H], F32, tag="xln", name="xln")
                    nc.vector.tensor_scalar(
                        xc[:], u4[:, m, :], rstd4b[:, m:m + 1],
                        negm4b[:, m:m + 1], op0=ALU.mult, op1=ALU.add,
                    )
                    nc.vector.tensor_mul(xc[:], xc[:], ln2g_bc[:, t, :])
                    uln = sb.tile([P, H], F32, tag="uln", name="uln")
                    nc.vector.tensor_add(uln[:], xc[:], ln2b_bc[:, t, :])
                    if last:
                        hf32 = sb.tile([P, H], F32, tag="hf32", name="hf32")
                        nc.scalar.activation(hf32[:], uln[:], AF.Gelu)
                        nc.sync.dma_start(hout[m * P:(m + 1) * P, :], hf32[:])
                    else:
                        nc.scalar.activation(h_nat_new[:, m, :], uln[:], AF.Gelu)
                if not last:
                    h_nat = h_nat_new

    nc.compile()
    return nc


_NC_CACHE = None


def _get_nc():
    global _NC_CACHE
    if _NC_CACHE is None:
        _NC_CACHE = _build()
    return _NC_CACHE


def _prepare_in_maps(inputs):
    ei = np.asarray(inputs["edge_index"])
    bond = np.asarray(inputs["bond_features"], dtype=np.float32)
    W_emb = np.asarray(inputs["W_emb"], dtype=np.float32)
    b_emb = np.asarray(inputs["b_emb"], dtype=np.float32)
    W_h = np.asarray(inputs["W_h"], dtype=np.float32)
    b_h = np.asarray(inputs["b_h"], dtype=np.float32)
    ln1_g = np.asarray(inputs["ln1_g"], dtype=np.float32)
    ln1_b = np.asarray(inputs["ln1_b"], dtype=np.float32)
    in_w = np.asarray(inputs["in_w"], dtype=np.float32)
    in_b = np.asarray(inputs["in_b"], dtype=np.float32)
    out_w = np.asarray(inputs["out_w"], dtype=np.float32)
    out_b = np.asarray(inputs["out_b"], dtype=np.float32)
    up_w = np.asarray(inputs["up_w"], dtype=np.float32)
    up_b = np.asarray(inputs["up_b"], dtype=np.float32)
    ln2_g = np.asarray(inputs["ln2_g"], dtype=np.float32)
    ln2_b = np.asarray(inputs["ln2_b"], dtype=np.float32)

    tgt = ei[1].astype(np.int64)
    deg = np.zeros(NN, np.float32)
    np.add.at(deg, tgt, 1.0)
    deg_tgt = deg[tgt]  # [E]

    # target-window permutation: core c owns edges with tgt//NW == c
    perm = np.argsort(tgt // NW, kind="stable")
    counts = np.bincount(tgt // NW, minlength=NC * 1)
    counts = np.bincount((tgt // NW).astype(np.int64), minlength=NC)
    assert counts.min() == counts.max() == EL, counts

    # fold LN1 gamma/beta into the in-projection (no 1/sqrt(d) here;
    # it lives in the exp constants / is neutral for V)
    in_w_s = in_w * ln1_g[:, :, None]
    in_b_s = in_b + np.einsum("lh,lho->lo", ln1_b, in_w)

    in_b_s[:, H:2 * H] = 0.0  # K bias cancels under softmax

    shared = {
        "wemb": W_emb.astype(BFNP),
        "bemb": b_emb.reshape(HK, P).T.copy(),
        "wh": W_h.reshape(HK, P, H).transpose(1, 0, 2).astype(BFNP),
        "bh": b_h.reshape(HK, P).T.copy(),
        "inw": in_w_s.reshape(L, HK, P, 3 * H).transpose(2, 0, 1, 3).astype(BFNP),
        "inb": in_b_s.reshape(L, M6, P).transpose(2, 0, 1).copy(),
        "outw": out_w.reshape(L, HK, P, H).transpose(2, 0, 1, 3).astype(BFNP),
        "upw": up_w.reshape(L, HK, P, H).transpose(2, 0, 1, 3).astype(BFNP),
        "upb2": (up_b + np.einsum("lh,lho->lo", out_b, up_w)).astype(np.float32),
        "ln2g": ln2_g, "ln2b": ln2_b,
        "inbv": np.ascontiguousarray(in_b_s[:, 2 * H:3 * H]),
    }
    # selector for the reciprocal broadcast: head h's den sits at staging
    # partition (h//2)*32; pair's first head covers cols 0:32, second 64:96
    PAIRS_H = ((0, 2), (1, 3), (4, 6), (5, 7))
    bs = np.zeros((P, 4, P), np.float32)
    for ihp, (hA, hB) in enumerate(PAIRS_H):
        bs[(hA // 2) * 32, ihp, 0:32] = 1.0
        bs[(hB // 2) * 32, ihp, 64:96] = 1.0
    shared["bselp"] = np.ascontiguousarray(bs.astype(BFNP))
    shared = {k: np.ascontiguousarray(v) for k, v in shared.items()}

    in_maps = []
    for c in range(NC):
        sl = perm[c * EL:(c + 1) * EL]
        tl = tgt[sl] - c * NW  # local node index in [0, NW)
        dl = deg_tgt[sl]
        A = np.zeros((EL, NW), np.float32)
        A[np.arange(EL), tl] = 1.0
        B = np.zeros((NW, EL), np.float32)
        B[tl, np.arange(EL)] = 1.0
        m = {
            "bondT": np.ascontiguousarray(bond[sl].T.astype(BFNP)),
            "Amat": np.ascontiguousarray(
                A.reshape(EC, P, NW).transpose(1, 0, 2).astype(BFNP)
            ),
            "Bmat": np.ascontiguousarray(
                B.reshape(NW, EC, P).astype(BFNP)
            ),
            "negdeg": np.ascontiguousarray(
                (-dl).reshape(EC, P).T.astype(np.float32)
            ),
        }
        m.update(shared)
        in_maps.append(m)
    return in_maps, perm


def kernel(**inputs):
    nc = _get_nc()
    in_maps, perm = _prepare_in_maps(inputs)
    res = run_bass_kernel_spmd(nc, in_maps, core_ids=list(range(NC)))
    out_perm = np.concatenate(
        [np.asarray(res.results[c]["hout"]) for c in range(NC)], axis=0
    )
    out = np.empty_like(out_perm)
    out[perm] = out_perm
    return out.astype(np.float32)
